# revision 11
# baseline (speedup 1.0000x reference)
"""Trainium2 Bass kernel for nn_Block2x2DiagProduct (butterfly product).

Strategy:
  Stages 1..9 of the butterfly (all with block size <= 512) compose into
  blockdiag(R, R) with a single dense 512x512 matrix R shared by both
  halves (parameters are shared across blocks within each factor). The
  final stage (block size 1024) is a columnwise 2x2 butterfly:

      out[:, k]     = A[k]*y[:, k] + B[k]*y[:, 512+k]
      out[:, 512+k] = C[k]*y[:, k] + D[k]*y[:, 512+k]

  where y = x @ blockdiag(R^T, R^T). So the device kernel is two K=512
  float32r matmuls per row tile (PE) plus a columnwise combine. This
  halves the PE matmul work vs composing one dense 1024x1024 matrix.

  R is composed on the host in float64. Sharding: pure data parallel —
  batch dim of x split across 8 cores; R^T (1 MiB) and the stage-0
  coefficients are replicated.

  Per-core pipeline, per 128-row tile of x:
    - HWDGE DMA in (f32 bits typed as float32r; PE rounds internally).
    - PE transposes the 8 [128,128] feature chunks 4-up into [128,512]
      PSUM tiles. Data and identity are float32r: f32r transposes run
      1.5 cyc/row vs 2.0 for plain f32 (walrus rejects mixing 32-bit
      f32r data with a 16-bit identity, so bf16's 1.0 is out of reach).
    - Scalar stages the transposes to SBUF; 8 accumulating f32r matmuls
      produce y_lo, y_hi in PSUM.
    - Stage-0 butterfly: Vector does 4 muls [128,512] against the
      (A|C) and (B|D) coefficient rows; GpSimd does ONE [128,1024] add
      (o = t02 + t13), then a per-tile 512 KiB HWDGE store.
  A burst of dummy PE transposes at t=0 starts the Tensor-engine
  p-state ramp early (cold PE runs matmuls at ~half clock for the
  first ~20 us otherwise), and the first two x loads are split into
  quarter/half transfers so real compute starts ~2 us sooner.
"""

import os
import sys

for _p in ("/opt/trn_rl_repo", "/root/.axon_site/_ro/trn_rl_repo"):
    if os.path.isdir(_p) and _p not in sys.path:
        sys.path.insert(0, _p)

import numpy as np

import concourse.bacc as bacc
import concourse.bass as bass
import concourse.mybir as mybir
from concourse.bass_utils import run_bass_kernel_spmd
from concourse.masks import make_identity
from concourse.tile import TileContext

SIZE = 1024
HALF = SIZE // 2
M = 10  # number of butterfly factors
N_CORES = 8
P = 128
KC = HALF // P  # 4 contraction chunks per half
N_WARM = 16  # dummy PE transposes to start the p-state ramp at t=0

# Results of the last device run (for the test harness).
last_exec_time_ns = None
last_mean_exec_time_ns = None

_nc_cache = {}


def _compose_w1t(params):
    """Compose butterfly stages 1..9 into W1t (512x512, f64) such that
    y_half = x_half @ W1t for each 512 half. Both halves share W1t because
    each factor's parameters are shared across its blocks."""
    w = np.eye(HALF, dtype=np.float64)
    for i in reversed(range(1, M)):
        s = SIZE >> i
        y = w.reshape(HALF, HALF // s, 2, s // 2)
        w = np.einsum(
            "ijk,bnjk->bnik", params[i].astype(np.float64), y
        ).reshape(HALF, HALF)
    return w


def _build_nc(rows):
    f32 = mybir.dt.float32
    f32r = mybir.dt.float32r
    nb = rows // P

    # Bacc (not raw Bass): its finalize() pipeline splits multi-sem waits
    # into EventSemaphore instructions (HW allows 1 sync-wait per inst).
    nc = bacc.Bacc(None, target_bir_lowering=False)
    x_d = nc.dram_tensor("x", [rows, SIZE], f32r, kind="ExternalInput")
    w_d = nc.dram_tensor("w", [HALF, HALF], f32, kind="ExternalInput")
    coef_d = nc.dram_tensor("coef", [P, 2, SIZE], f32, kind="ExternalInput")
    o_d = nc.dram_tensor("o", [rows, SIZE], f32, kind="ExternalOutput")

    with TileContext(nc) as tc:
        with (
            tc.tile_pool(name="const", bufs=1) as const_pool,
            tc.tile_pool(name="xin", bufs=8) as xpool,
            tc.tile_pool(name="xt", bufs=4) as xtpool,
            tc.tile_pool(name="tmul", bufs=6) as tpool,
            tc.tile_pool(name="osb", bufs=6) as opool,
            tc.tile_pool(name="tpsum", bufs=2, space="PSUM") as tpsum,
            tc.tile_pool(name="mpsum", bufs=4, space="PSUM") as mpsum,
        ):
            # GpSimd memset/affine_select reject f32r tiles, so build the
            # identity in f32 and cast (bit-copy) to the f32r operand the
            # transposes use. The warmup source reuses the f32r identity.
            ident_f32 = const_pool.tile([P, P], f32)
            make_identity(nc, ident_f32[:])
            ident = const_pool.tile([P, P], f32r)
            nc.vector.tensor_copy(out=ident[:], in_=ident_f32[:])
            warm_src = ident
            # PE p-state warmup: back-to-back dummy transposes from t~0.5us
            # keep the Tensor engine continuously busy so it reaches full
            # clock before the first real tile arrives (cold PE runs at
            # ~1.2 GHz, halving matmul throughput for ~20 us otherwise).
            # Also absorbs the one-sync-wait rule for the first real
            # transpose (identity is already consumed on-engine).
            pwarm = tpsum.tile([P, P], f32r, name="pwarm", tag="pst")
            for _ in range(N_WARM):
                nc.tensor.transpose(pwarm[:], warm_src[:], ident[:])

            # W1t resident in SBUF: partition p, chunk c holds W1t[c*128+p, :].
            # Per-chunk loads on the ACT HWDGE queue run in parallel with the
            # x loads on the SP queue; chunk 0's float32r cast is ready early.
            w_sb = const_pool.tile([P, KC, HALF], f32)
            w_sbr = const_pool.tile([P, KC, HALF], f32r)
            for c in range(KC):
                nc.scalar.dma_start(
                    out=w_sb[:, c, :], in_=w_d[c * P : (c + 1) * P, :]
                )
                # FP32r matmul operands must be produced rounded-to-FP32r.
                nc.vector.tensor_copy(out=w_sbr[:, c, :], in_=w_sb[:, c, :])
            # Stage-0 coefficients as (A|C) and (B|D) rows, pre-replicated
            # across partitions: o_lo|o_hi = (A|C)*y_lo_2 + (B|D)*y_hi_2.
            coef_sb = const_pool.tile([P, 2, SIZE], f32)
            nc.scalar.dma_start(out=coef_sb[:], in_=coef_d[:, :, :])

            # Store issues are deferred by two tiles: a store's semaphore
            # wait (on its GpSimd add) otherwise head-of-line blocks the
            # in-order Scalar sequencer and delays the next tile's PSUM->
            # SBUF staging copy, stalling PE. Two tiles (~5.6 us) is past
            # the add's completion, so the deferred wait is already
            # satisfied when the sequencer reaches it.
            pending_stores = []
            for bp in range(nb // 2):
                # Two 128-row tiles per DMA: 1 MiB transfers are the DMA
                # bandwidth sweet spot. The first two loads are split into
                # quarter/half transfers so the first transposes can start
                # ~2 us earlier, shortening the pipeline ramp.
                x_sb = xpool.tile([P, 2, SIZE], f32r)
                if bp == 0:
                    for j in range(2):
                        for hh in range(2):
                            nc.sync.dma_start(
                                out=x_sb[:, j, hh * HALF : (hh + 1) * HALF],
                                in_=x_d[
                                    (bp * 2 + j) * P : (bp * 2 + j + 1) * P,
                                    hh * HALF : (hh + 1) * HALF,
                                ],
                            )
                elif bp == 1:
                    for j in range(2):
                        nc.sync.dma_start(
                            out=x_sb[:, j, :],
                            in_=x_d[
                                (bp * 2 + j) * P : (bp * 2 + j + 1) * P, :
                            ],
                        )
                else:
                    nc.sync.dma_start(
                        out=x_sb[:],
                        in_=x_d[bp * 2 * P : (bp + 1) * 2 * P, :].rearrange(
                            "(j p) f -> p j f", p=P
                        ),
                    )
                for j in range(2):
                    # Transpose 8 chunks of [128b, 128f] -> [128f, 128b]
                    # into one 2-bank PSUM tile (each individual transpose
                    # stays within a single bank), then ONE [128,1024]
                    # Scalar-engine copy stages all of x^T to SBUF.
                    pst = tpsum.tile([P, 2, HALF], f32r, tag="pst", name="pst")
                    for h in range(2):
                        for c in range(KC):
                            k = KC * h + c
                            nc.tensor.transpose(
                                pst[:, h, c * P : (c + 1) * P],
                                x_sb[:, j, k * P : (k + 1) * P],
                                ident[:],
                            )
                    xt = xtpool.tile([P, 2, HALF], f32r, tag="xt", name="xt")
                    nc.scalar.copy(out=xt[:], in_=pst[:])
                    # y_half[b, :] = sum_k x_half[b, k] * W1t[k, :]
                    psos = [
                        mpsum.tile([P, HALF], f32, tag="mm_psum", name=f"pso{h}")
                        for h in range(2)
                    ]
                    for c in range(KC):
                        for h in range(2):
                            nc.tensor.matmul(
                                psos[h][:],
                                xt[:, h, c * P : (c + 1) * P],
                                w_sbr[:, c, :],
                                start=(c == 0),
                                stop=(c == KC - 1),
                            )
                    # Peeled stage 0: Vector computes t02 = (A|C)*y_lo and
                    # t13 = (B|D)*y_hi straight from PSUM (GpSimd cannot
                    # read PSUM); GpSimd does one [128,1024] add into the
                    # output tile, which gates the per-tile store.
                    t02 = tpool.tile([P, SIZE], f32, tag="t02", name="t02")
                    t13 = tpool.tile([P, SIZE], f32, tag="t13", name="t13")
                    nc.vector.tensor_mul(
                        t02[:, :HALF], psos[0][:], coef_sb[:, 0, :HALF]
                    )
                    nc.vector.tensor_mul(
                        t02[:, HALF:], psos[0][:], coef_sb[:, 0, HALF:]
                    )
                    nc.vector.tensor_mul(
                        t13[:, :HALF], psos[1][:], coef_sb[:, 1, :HALF]
                    )
                    nc.vector.tensor_mul(
                        t13[:, HALF:], psos[1][:], coef_sb[:, 1, HALF:]
                    )
                    o_sb = opool.tile([P, SIZE], f32, tag="osb", name="osb")
                    nc.gpsimd.tensor_add(o_sb[:], t02[:], t13[:])
                    # Per-tile 512 KiB store on the ACT HWDGE queue: loads
                    # (SP queue) and stores stream through separate DMA
                    # queues, and each store only waits on its own tile's
                    # single GpSimd add (short drain tail).
                    pending_stores.append((o_sb, (bp * 2 + j) * P))
                    if len(pending_stores) > 2:
                        o_prev, row0 = pending_stores.pop(0)
                        nc.scalar.dma_start(
                            out=o_d[row0 : row0 + P, :], in_=o_prev[:]
                        )
            for o_prev, row0 in pending_stores:
                nc.scalar.dma_start(out=o_d[row0 : row0 + P, :], in_=o_prev[:])
    nc.finalize()
    return nc


def kernel(**inputs):
    global last_exec_time_ns, last_mean_exec_time_ns

    x = np.ascontiguousarray(np.asarray(inputs["x"], dtype=np.float32))
    params = [np.asarray(inputs[f"ABCD{i}"]) for i in range(M)]
    w1t = np.ascontiguousarray(_compose_w1t(params).astype(np.float32))
    abcd0 = params[0].astype(np.float32)  # (2, 2, 512)
    # coef[:, 0, :] = (A|C), coef[:, 1, :] = (B|D), replicated across
    # partitions: o = coef0 * (y_lo|y_lo) + coef1 * (y_hi|y_hi).
    coef_rows = np.stack(
        [
            np.concatenate([abcd0[0, 0], abcd0[1, 0]]),
            np.concatenate([abcd0[0, 1], abcd0[1, 1]]),
        ]
    )  # (2, 1024)
    coef = np.ascontiguousarray(
        np.broadcast_to(coef_rows[None], (P, 2, SIZE)).astype(np.float32)
    )

    batch = x.shape[0]
    if batch % (N_CORES * 2 * P) != 0:
        # Shape outside the tiled layout this kernel hardcodes — fall back
        # to a host matmul (correct, just not accelerated).
        full = _compose_w1t(params)
        y_lo = x[:, :HALF].astype(np.float64) @ full
        y_hi = x[:, HALF:].astype(np.float64) @ full
        a, b = params[0][0, 0].astype(np.float64), params[0][0, 1].astype(
            np.float64
        )
        c, dd = params[0][1, 0].astype(np.float64), params[0][1, 1].astype(
            np.float64
        )
        return np.concatenate(
            [a * y_lo + b * y_hi, c * y_lo + dd * y_hi], axis=1
        ).astype(np.float32)
    rows = batch // N_CORES

    if rows not in _nc_cache:
        _nc_cache[rows] = _build_nc(rows)
    nc = _nc_cache[rows]

    in_maps = [
        {"x": x[i * rows : (i + 1) * rows], "w": w1t, "coef": coef}
        for i in range(N_CORES)
    ]
    try:
        res = run_bass_kernel_spmd(nc, in_maps, core_ids=list(range(N_CORES)))
    except Exception:
        # Transient axon/PJRT INTERNAL errors have been observed on the
        # first attempt in a fresh process; one retry clears them.
        res = run_bass_kernel_spmd(nc, in_maps, core_ids=list(range(N_CORES)))
    last_exec_time_ns = res.exec_time_ns
    last_mean_exec_time_ns = res.mean_exec_time_ns

    return np.concatenate([r["o"] for r in res.results], axis=0)


# revision 13
# speedup vs baseline: 1.1173x; 1.1173x over previous
"""Trainium2 Bass kernel for nn_Block2x2DiagProduct (butterfly product).

Strategy:
  Stages 1..9 of the butterfly (all with block size <= 512) compose into
  blockdiag(R, R) with a single dense 512x512 matrix R shared by both
  halves (parameters are shared across blocks within each factor). The
  final stage (block size 1024) is a columnwise 2x2 butterfly:

      out[:, k]     = A[k]*y[:, k] + B[k]*y[:, 512+k]
      out[:, 512+k] = C[k]*y[:, k] + D[k]*y[:, 512+k]

  where y = x @ blockdiag(R^T, R^T). So the device kernel is two K=512
  float32r matmuls per row tile (PE) plus a columnwise combine. This
  halves the PE matmul work vs composing one dense 1024x1024 matrix.

  R is composed on the host in float64. Sharding: pure data parallel —
  batch dim of x split across 8 cores; R^T (1 MiB) and the stage-0
  coefficients are replicated.

  Per-core pipeline, per 128-row tile of x:
    - HWDGE DMA in (f32 bits typed as float32r; PE rounds internally).
    - PE transposes the 8 [128,128] feature chunks 4-up into [128,512]
      PSUM tiles. Data and identity are float32r: f32r transposes run
      1.5 cyc/row vs 2.0 for plain f32 (walrus rejects mixing 32-bit
      f32r data with a 16-bit identity, so bf16's 1.0 is out of reach).
    - Scalar stages the transposes to SBUF; 8 accumulating f32r matmuls
      produce y_lo, y_hi in PSUM.
    - Stage-0 butterfly: Vector does 4 muls [128,512] against the
      (A|C) and (B|D) coefficient rows; GpSimd does ONE [128,1024] add
      (o = t02 + t13), then a per-tile 512 KiB HWDGE store.
  A burst of dummy PE transposes at t=0 starts the Tensor-engine
  p-state ramp early (cold PE runs matmuls at ~half clock for the
  first ~20 us otherwise), and the first two x loads are split into
  quarter/half transfers so real compute starts ~2 us sooner.
"""

import os
import sys

for _p in ("/opt/trn_rl_repo", "/root/.axon_site/_ro/trn_rl_repo"):
    if os.path.isdir(_p) and _p not in sys.path:
        sys.path.insert(0, _p)

import numpy as np

import concourse.bacc as bacc
import concourse.bass as bass
import concourse.mybir as mybir
from concourse.bass_utils import run_bass_kernel_spmd
from concourse.masks import make_identity
from concourse.tile import TileContext

SIZE = 1024
HALF = SIZE // 2
M = 10  # number of butterfly factors
N_CORES = 8
P = 128
KC = HALF // P  # 4 contraction chunks per half
N_WARM = 16  # dummy PE transposes to start the p-state ramp at t=0

# Results of the last device run (for the test harness).
last_exec_time_ns = None
last_mean_exec_time_ns = None

_nc_cache = {}


def _compose_w1t(params):
    """Compose butterfly stages 1..9 into W1t (512x512, f64) such that
    y_half = x_half @ W1t for each 512 half. Both halves share W1t because
    each factor's parameters are shared across its blocks."""
    w = np.eye(HALF, dtype=np.float64)
    for i in reversed(range(1, M)):
        s = SIZE >> i
        y = w.reshape(HALF, HALF // s, 2, s // 2)
        w = np.einsum(
            "ijk,bnjk->bnik", params[i].astype(np.float64), y
        ).reshape(HALF, HALF)
    return w


def _build_nc(rows):
    f32 = mybir.dt.float32
    f32r = mybir.dt.float32r
    nb = rows // P

    # Bacc (not raw Bass): its finalize() pipeline splits multi-sem waits
    # into EventSemaphore instructions (HW allows 1 sync-wait per inst).
    nc = bacc.Bacc(None, target_bir_lowering=False)
    x_d = nc.dram_tensor("x", [rows, SIZE], f32r, kind="ExternalInput")
    w_d = nc.dram_tensor("w", [HALF, HALF], f32, kind="ExternalInput")
    coef_d = nc.dram_tensor("coef", [P, 2, SIZE], f32, kind="ExternalInput")
    o_d = nc.dram_tensor("o", [rows, SIZE], f32, kind="ExternalOutput")

    with TileContext(nc) as tc:
        with (
            tc.tile_pool(name="const", bufs=1) as const_pool,
            tc.tile_pool(name="xin", bufs=8) as xpool,
            tc.tile_pool(name="xt", bufs=4) as xtpool,
            tc.tile_pool(name="tmul", bufs=6) as tpool,
            tc.tile_pool(name="osb", bufs=6) as opool,
            tc.tile_pool(name="tpsum", bufs=4, space="PSUM") as tpsum,
            tc.tile_pool(name="mpsum", bufs=4, space="PSUM") as mpsum,
        ):
            # GpSimd memset/affine_select reject f32r tiles, so build the
            # identity in f32 and cast (bit-copy) to the f32r operand the
            # transposes use. The warmup source reuses the f32r identity.
            ident_f32 = const_pool.tile([P, P], f32)
            make_identity(nc, ident_f32[:])
            ident = const_pool.tile([P, P], f32r)
            nc.vector.tensor_copy(out=ident[:], in_=ident_f32[:])
            warm_src = ident
            # PE p-state warmup: back-to-back dummy transposes from t~0.5us
            # keep the Tensor engine continuously busy so it reaches full
            # clock before the first real tile arrives (cold PE runs at
            # ~1.2 GHz, halving matmul throughput for ~20 us otherwise).
            # Also absorbs the one-sync-wait rule for the first real
            # transpose (identity is already consumed on-engine).
            pwarm = tpsum.tile([P, P], f32r, name="pwarm", tag="pst")
            for _ in range(N_WARM):
                nc.tensor.transpose(pwarm[:], warm_src[:], ident[:])

            # W1t resident in SBUF: partition p, chunk c holds W1t[c*128+p, :].
            # Per-chunk loads on the ACT HWDGE queue run in parallel with the
            # x loads on the SP queue; chunk 0's float32r cast is ready early.
            w_sb = const_pool.tile([P, KC, HALF], f32)
            w_sbr = const_pool.tile([P, KC, HALF], f32r)
            for c in range(KC):
                nc.scalar.dma_start(
                    out=w_sb[:, c, :], in_=w_d[c * P : (c + 1) * P, :]
                )
                # FP32r matmul operands must be produced rounded-to-FP32r.
                nc.vector.tensor_copy(out=w_sbr[:, c, :], in_=w_sb[:, c, :])
            # Stage-0 coefficients as (A|C) and (B|D) rows, pre-replicated
            # across partitions: o_lo|o_hi = (A|C)*y_lo_2 + (B|D)*y_hi_2.
            coef_sb = const_pool.tile([P, 2, SIZE], f32)
            nc.scalar.dma_start(out=coef_sb[:], in_=coef_d[:, :, :])

            # Store issues are deferred by two tiles: a store's semaphore
            # wait (on its GpSimd add) otherwise head-of-line blocks the
            # in-order Scalar sequencer and delays the next tile's PSUM->
            # SBUF staging copy, stalling PE. Two tiles (~5 us) is past
            # the add's completion, so the deferred wait is already
            # satisfied when the sequencer reaches it.
            pending_stores = []

            def emit_store(force=False):
                while pending_stores and (force or len(pending_stores) > 2):
                    o_prev, row0 = pending_stores.pop(0)
                    nc.scalar.dma_start(
                        out=o_d[row0 : row0 + P, :], in_=o_prev[:]
                    )

            # The per-tile compute chain is software-pipelined one tile
            # deep at EMISSION level: transposes for tile t+1 are emitted
            # before the matmuls of tile t, so the in-order PE stream
            # never sits waiting for tile t's PSUM->SBUF staging copies —
            # it transposes the next tile instead.
            def emit_compute(xts, row0):
                psos = [
                    mpsum.tile([P, HALF], f32, tag="mm_psum", name=f"pso{h}")
                    for h in range(2)
                ]
                for c in range(KC):
                    for h in range(2):
                        nc.tensor.matmul(
                            psos[h][:],
                            xts[h][:, c * P : (c + 1) * P],
                            w_sbr[:, c, :],
                            start=(c == 0),
                            stop=(c == KC - 1),
                        )
                # Peeled stage 0: Vector computes t02 = (A|C)*y_lo and
                # t13 = (B|D)*y_hi straight from PSUM (GpSimd cannot
                # read PSUM); GpSimd does one [128,1024] add into the
                # output tile, which gates the per-tile 512 KiB store.
                t02 = tpool.tile([P, SIZE], f32, tag="t02", name="t02")
                t13 = tpool.tile([P, SIZE], f32, tag="t13", name="t13")
                nc.vector.tensor_mul(
                    t02[:, :HALF], psos[0][:], coef_sb[:, 0, :HALF]
                )
                nc.vector.tensor_mul(
                    t02[:, HALF:], psos[0][:], coef_sb[:, 0, HALF:]
                )
                nc.vector.tensor_mul(
                    t13[:, :HALF], psos[1][:], coef_sb[:, 1, :HALF]
                )
                nc.vector.tensor_mul(
                    t13[:, HALF:], psos[1][:], coef_sb[:, 1, HALF:]
                )
                o_sb = opool.tile([P, SIZE], f32, tag="osb", name="osb")
                nc.gpsimd.tensor_add(o_sb[:], t02[:], t13[:])
                pending_stores.append((o_sb, row0))
                emit_store()

            pending_compute = None
            x_sb = None
            for t in range(nb):
                bp, j = divmod(t, 2)
                if j == 0:
                    # Two 128-row tiles per DMA: 1 MiB transfers are the
                    # DMA bandwidth sweet spot. The first two loads are
                    # split into quarter/half transfers so the first
                    # transposes start ~2 us earlier (shorter ramp).
                    x_sb = xpool.tile([P, 2, SIZE], f32r)
                    if bp == 0:
                        for jj in range(2):
                            for hh in range(2):
                                nc.sync.dma_start(
                                    out=x_sb[
                                        :, jj, hh * HALF : (hh + 1) * HALF
                                    ],
                                    in_=x_d[
                                        (bp * 2 + jj) * P : (bp * 2 + jj + 1)
                                        * P,
                                        hh * HALF : (hh + 1) * HALF,
                                    ],
                                )
                    elif bp == 1:
                        for jj in range(2):
                            nc.sync.dma_start(
                                out=x_sb[:, jj, :],
                                in_=x_d[
                                    (bp * 2 + jj) * P : (bp * 2 + jj + 1) * P,
                                    :,
                                ],
                            )
                    else:
                        nc.sync.dma_start(
                            out=x_sb[:],
                            in_=x_d[
                                bp * 2 * P : (bp + 1) * 2 * P, :
                            ].rearrange("(j p) f -> p j f", p=P),
                        )
                # Transpose 8 chunks of [128b, 128f] -> [128f, 128b],
                # 4 chunks per PSUM bank, one Scalar-engine copy per half.
                xts = []
                for h in range(2):
                    pst = tpsum.tile([P, HALF], f32r, tag="pst", name=f"pst{h}")
                    for c in range(KC):
                        k = KC * h + c
                        nc.tensor.transpose(
                            pst[:, c * P : (c + 1) * P],
                            x_sb[:, j, k * P : (k + 1) * P],
                            ident[:],
                        )
                    xt_h = xtpool.tile([P, HALF], f32r, tag="xt", name=f"xt{h}")
                    nc.scalar.copy(out=xt_h[:], in_=pst[:])
                    xts.append(xt_h)
                if pending_compute is not None:
                    emit_compute(*pending_compute)
                pending_compute = (xts, t * P)
            emit_compute(*pending_compute)
            emit_store(force=True)
    nc.finalize()
    return nc


def kernel(**inputs):
    global last_exec_time_ns, last_mean_exec_time_ns

    x = np.ascontiguousarray(np.asarray(inputs["x"], dtype=np.float32))
    params = [np.asarray(inputs[f"ABCD{i}"]) for i in range(M)]
    w1t = np.ascontiguousarray(_compose_w1t(params).astype(np.float32))
    abcd0 = params[0].astype(np.float32)  # (2, 2, 512)
    # coef[:, 0, :] = (A|C), coef[:, 1, :] = (B|D), replicated across
    # partitions: o = coef0 * (y_lo|y_lo) + coef1 * (y_hi|y_hi).
    coef_rows = np.stack(
        [
            np.concatenate([abcd0[0, 0], abcd0[1, 0]]),
            np.concatenate([abcd0[0, 1], abcd0[1, 1]]),
        ]
    )  # (2, 1024)
    coef = np.ascontiguousarray(
        np.broadcast_to(coef_rows[None], (P, 2, SIZE)).astype(np.float32)
    )

    batch = x.shape[0]
    if batch % (N_CORES * 2 * P) != 0:
        # Shape outside the tiled layout this kernel hardcodes — fall back
        # to a host matmul (correct, just not accelerated).
        full = _compose_w1t(params)
        y_lo = x[:, :HALF].astype(np.float64) @ full
        y_hi = x[:, HALF:].astype(np.float64) @ full
        a, b = params[0][0, 0].astype(np.float64), params[0][0, 1].astype(
            np.float64
        )
        c, dd = params[0][1, 0].astype(np.float64), params[0][1, 1].astype(
            np.float64
        )
        return np.concatenate(
            [a * y_lo + b * y_hi, c * y_lo + dd * y_hi], axis=1
        ).astype(np.float32)
    rows = batch // N_CORES

    if rows not in _nc_cache:
        _nc_cache[rows] = _build_nc(rows)
    nc = _nc_cache[rows]

    in_maps = [
        {"x": x[i * rows : (i + 1) * rows], "w": w1t, "coef": coef}
        for i in range(N_CORES)
    ]
    try:
        res = run_bass_kernel_spmd(nc, in_maps, core_ids=list(range(N_CORES)))
    except Exception:
        # Transient axon/PJRT INTERNAL errors have been observed on the
        # first attempt in a fresh process; one retry clears them.
        res = run_bass_kernel_spmd(nc, in_maps, core_ids=list(range(N_CORES)))
    last_exec_time_ns = res.exec_time_ns
    last_mean_exec_time_ns = res.mean_exec_time_ns

    return np.concatenate([r["o"] for r in res.results], axis=0)


# revision 14
# speedup vs baseline: 1.1293x; 1.0107x over previous
"""Trainium2 Bass kernel for nn_Block2x2DiagProduct (butterfly product).

Strategy:
  Stages 1..9 of the butterfly (all with block size <= 512) compose into
  blockdiag(R, R) with a single dense 512x512 matrix R shared by both
  halves (parameters are shared across blocks within each factor). The
  final stage (block size 1024) is a columnwise 2x2 butterfly:

      out[:, k]     = A[k]*y[:, k] + B[k]*y[:, 512+k]
      out[:, 512+k] = C[k]*y[:, k] + D[k]*y[:, 512+k]

  where y = x @ blockdiag(R^T, R^T). So the device kernel is two K=512
  float32r matmuls per row tile (PE) plus a columnwise combine. This
  halves the PE matmul work vs composing one dense 1024x1024 matrix.

  R is composed on the host in float64. Sharding: pure data parallel —
  batch dim of x split across 8 cores; R^T (1 MiB) and the stage-0
  coefficients are replicated.

  Per-core pipeline, per 128-row tile of x:
    - HWDGE DMA in (f32 bits typed as float32r; PE rounds internally).
    - PE transposes the 8 [128,128] feature chunks 4-up into [128,512]
      PSUM tiles. Data and identity are float32r: f32r transposes run
      1.5 cyc/row vs 2.0 for plain f32 (walrus rejects mixing 32-bit
      f32r data with a 16-bit identity, so bf16's 1.0 is out of reach).
    - Scalar stages the transposes to SBUF; 8 accumulating f32r matmuls
      produce y_lo, y_hi in PSUM.
    - Stage-0 butterfly: Vector does 4 muls [128,512] against the
      (A|C) and (B|D) coefficient rows; GpSimd does ONE [128,1024] add
      (o = t02 + t13), then a per-tile 512 KiB HWDGE store.
  A burst of dummy PE transposes at t=0 starts the Tensor-engine
  p-state ramp early (cold PE runs matmuls at ~half clock for the
  first ~20 us otherwise), and the first two x loads are split into
  quarter/half transfers so real compute starts ~2 us sooner.
"""

import os
import sys

for _p in ("/opt/trn_rl_repo", "/root/.axon_site/_ro/trn_rl_repo"):
    if os.path.isdir(_p) and _p not in sys.path:
        sys.path.insert(0, _p)

import numpy as np

import concourse.bacc as bacc
import concourse.bass as bass
import concourse.mybir as mybir
from concourse.bass_utils import run_bass_kernel_spmd
from concourse.masks import make_identity
from concourse.tile import TileContext

SIZE = 1024
HALF = SIZE // 2
M = 10  # number of butterfly factors
N_CORES = 8
P = 128
KC = HALF // P  # 4 contraction chunks per half
N_WARM = 16  # dummy PE transposes to start the p-state ramp at t=0

# Results of the last device run (for the test harness).
last_exec_time_ns = None
last_mean_exec_time_ns = None

_nc_cache = {}


def _compose_w1t(params):
    """Compose butterfly stages 1..9 into W1t (512x512, f64) such that
    y_half = x_half @ W1t for each 512 half. Both halves share W1t because
    each factor's parameters are shared across its blocks."""
    w = np.eye(HALF, dtype=np.float64)
    for i in reversed(range(1, M)):
        s = SIZE >> i
        y = w.reshape(HALF, HALF // s, 2, s // 2)
        w = np.einsum(
            "ijk,bnjk->bnik", params[i].astype(np.float64), y
        ).reshape(HALF, HALF)
    return w


def _build_nc(rows):
    f32 = mybir.dt.float32
    f32r = mybir.dt.float32r
    nb = rows // P

    # Bacc (not raw Bass): its finalize() pipeline splits multi-sem waits
    # into EventSemaphore instructions (HW allows 1 sync-wait per inst).
    nc = bacc.Bacc(None, target_bir_lowering=False)
    x_d = nc.dram_tensor("x", [rows, SIZE], f32r, kind="ExternalInput")
    w_d = nc.dram_tensor("w", [HALF, HALF], f32, kind="ExternalInput")
    coef_d = nc.dram_tensor("coef", [P, 2, SIZE], f32, kind="ExternalInput")
    o_d = nc.dram_tensor("o", [rows, SIZE], f32, kind="ExternalOutput")

    with TileContext(nc) as tc:
        with (
            tc.tile_pool(name="const", bufs=1) as const_pool,
            tc.tile_pool(name="xin", bufs=8) as xpool,
            tc.tile_pool(name="xt", bufs=4) as xtpool,
            tc.tile_pool(name="tmul", bufs=6) as tpool,
            tc.tile_pool(name="osb", bufs=6) as opool,
            tc.tile_pool(name="tpsum", bufs=4, space="PSUM") as tpsum,
            tc.tile_pool(name="mpsum", bufs=4, space="PSUM") as mpsum,
        ):
            # GpSimd memset/affine_select reject f32r tiles, so build the
            # identity in f32 and cast (bit-copy) to the f32r operand the
            # transposes use. The warmup source reuses the f32r identity.
            ident_f32 = const_pool.tile([P, P], f32)
            make_identity(nc, ident_f32[:])
            ident = const_pool.tile([P, P], f32r)
            nc.vector.tensor_copy(out=ident[:], in_=ident_f32[:])
            warm_src = ident
            # PE p-state warmup: back-to-back dummy transposes from t~0.5us
            # keep the Tensor engine continuously busy so it reaches full
            # clock before the first real tile arrives (cold PE runs at
            # ~1.2 GHz, halving matmul throughput for ~20 us otherwise).
            # Also absorbs the one-sync-wait rule for the first real
            # transpose (identity is already consumed on-engine).
            pwarm = tpsum.tile([P, P], f32r, name="pwarm", tag="pst")
            for _ in range(N_WARM):
                nc.tensor.transpose(pwarm[:], warm_src[:], ident[:])

            # W1t resident in SBUF: partition p, chunk c holds W1t[c*128+p, :].
            # Per-chunk loads on the ACT HWDGE queue run in parallel with the
            # x loads on the SP queue; chunk 0's float32r cast is ready early.
            w_sb = const_pool.tile([P, KC, HALF], f32)
            w_sbr = const_pool.tile([P, KC, HALF], f32r)
            for c in range(KC):
                nc.scalar.dma_start(
                    out=w_sb[:, c, :], in_=w_d[c * P : (c + 1) * P, :]
                )
                # FP32r matmul operands must be produced rounded-to-FP32r.
                nc.vector.tensor_copy(out=w_sbr[:, c, :], in_=w_sb[:, c, :])
            # Stage-0 coefficients as (A|C) and (B|D) rows, pre-replicated
            # across partitions: o_lo|o_hi = (A|C)*y_lo_2 + (B|D)*y_hi_2.
            coef_sb = const_pool.tile([P, 2, SIZE], f32)
            nc.scalar.dma_start(out=coef_sb[:], in_=coef_d[:, :, :])

            # Store issues are deferred by two tiles: a store's semaphore
            # wait (on its GpSimd add) otherwise head-of-line blocks the
            # in-order Scalar sequencer and delays the next tile's PSUM->
            # SBUF staging copy, stalling PE. Two tiles (~5 us) is past
            # the add's completion, so the deferred wait is already
            # satisfied when the sequencer reaches it.
            pending_stores = []

            def emit_store(force=False):
                while pending_stores and (force or len(pending_stores) > 2):
                    o_prev, row0 = pending_stores.pop(0)
                    nc.scalar.dma_start(
                        out=o_d[row0 : row0 + P, :], in_=o_prev[:]
                    )

            # The per-tile compute chain is software-pipelined one tile
            # deep at EMISSION level: transposes for tile t+1 are emitted
            # before the matmuls of tile t, so the in-order PE stream
            # never sits waiting for tile t's PSUM->SBUF staging copies —
            # it transposes the next tile instead.
            def emit_compute(xts, row0):
                psos = [
                    mpsum.tile([P, HALF], f32, tag="mm_psum", name=f"pso{h}")
                    for h in range(2)
                ]
                for c in range(KC):
                    for h in range(2):
                        nc.tensor.matmul(
                            psos[h][:],
                            xts[h][:, c * P : (c + 1) * P],
                            w_sbr[:, c, :],
                            start=(c == 0),
                            stop=(c == KC - 1),
                        )
                # Peeled stage 0: Vector computes t02 = (A|C)*y_lo and
                # t13 = (B|D)*y_hi straight from PSUM (GpSimd cannot
                # read PSUM); GpSimd does one [128,1024] add into the
                # output tile, which gates the per-tile 512 KiB store.
                t02 = tpool.tile([P, SIZE], f32, tag="t02", name="t02")
                t13 = tpool.tile([P, SIZE], f32, tag="t13", name="t13")
                nc.vector.tensor_mul(
                    t02[:, :HALF], psos[0][:], coef_sb[:, 0, :HALF]
                )
                nc.vector.tensor_mul(
                    t02[:, HALF:], psos[0][:], coef_sb[:, 0, HALF:]
                )
                nc.vector.tensor_mul(
                    t13[:, :HALF], psos[1][:], coef_sb[:, 1, :HALF]
                )
                nc.vector.tensor_mul(
                    t13[:, HALF:], psos[1][:], coef_sb[:, 1, HALF:]
                )
                o_sb = opool.tile([P, SIZE], f32, tag="osb", name="osb")
                nc.gpsimd.tensor_add(o_sb[:], t02[:], t13[:])
                pending_stores.append((o_sb, row0))
                emit_store()

            pending_compute = None
            x_sb = None
            for t in range(nb):
                bp, j = divmod(t, 2)
                if j == 0:
                    # Two 128-row tiles per DMA: 1 MiB transfers are the
                    # DMA bandwidth sweet spot. The first two loads are
                    # split into quarter/half transfers so the first
                    # transposes start ~2 us earlier (shorter ramp).
                    x_sb = xpool.tile([P, 2, SIZE], f32r)
                    if bp == 0:
                        for jj in range(2):
                            for hh in range(2):
                                nc.sync.dma_start(
                                    out=x_sb[
                                        :, jj, hh * HALF : (hh + 1) * HALF
                                    ],
                                    in_=x_d[
                                        (bp * 2 + jj) * P : (bp * 2 + jj + 1)
                                        * P,
                                        hh * HALF : (hh + 1) * HALF,
                                    ],
                                )
                    elif bp == 1:
                        for jj in range(2):
                            nc.sync.dma_start(
                                out=x_sb[:, jj, :],
                                in_=x_d[
                                    (bp * 2 + jj) * P : (bp * 2 + jj + 1) * P,
                                    :,
                                ],
                            )
                    else:
                        nc.sync.dma_start(
                            out=x_sb[:],
                            in_=x_d[
                                bp * 2 * P : (bp + 1) * 2 * P, :
                            ].rearrange("(j p) f -> p j f", p=P),
                        )
                # Transpose 8 chunks of [128b, 128f] -> [128f, 128b],
                # 4 chunks per PSUM bank, one Scalar-engine copy per half.
                # The copies are emitted AFTER the previous tile's compute:
                # matmul wait thresholds coarsen to the latest Scalar op
                # emitted before them, so emitting copies(t) after mm(t-1)
                # keeps mm(t)'s effective wait on copies(t) (correct, and
                # long done) instead of copies(t+1) (a ~1 us PE stall per
                # tile observed when copies precede the compute emission).
                psts = []
                for h in range(2):
                    pst = tpsum.tile([P, HALF], f32r, tag="pst", name=f"pst{h}")
                    for c in range(KC):
                        k = KC * h + c
                        nc.tensor.transpose(
                            pst[:, c * P : (c + 1) * P],
                            x_sb[:, j, k * P : (k + 1) * P],
                            ident[:],
                        )
                    psts.append(pst)
                if pending_compute is not None:
                    emit_compute(*pending_compute)
                xts = []
                for h in range(2):
                    xt_h = xtpool.tile([P, HALF], f32r, tag="xt", name=f"xt{h}")
                    nc.scalar.copy(out=xt_h[:], in_=psts[h][:])
                    xts.append(xt_h)
                pending_compute = (xts, t * P)
            emit_compute(*pending_compute)
            emit_store(force=True)
    nc.finalize()
    return nc


def kernel(**inputs):
    global last_exec_time_ns, last_mean_exec_time_ns

    x = np.ascontiguousarray(np.asarray(inputs["x"], dtype=np.float32))
    params = [np.asarray(inputs[f"ABCD{i}"]) for i in range(M)]
    w1t = np.ascontiguousarray(_compose_w1t(params).astype(np.float32))
    abcd0 = params[0].astype(np.float32)  # (2, 2, 512)
    # coef[:, 0, :] = (A|C), coef[:, 1, :] = (B|D), replicated across
    # partitions: o = coef0 * (y_lo|y_lo) + coef1 * (y_hi|y_hi).
    coef_rows = np.stack(
        [
            np.concatenate([abcd0[0, 0], abcd0[1, 0]]),
            np.concatenate([abcd0[0, 1], abcd0[1, 1]]),
        ]
    )  # (2, 1024)
    coef = np.ascontiguousarray(
        np.broadcast_to(coef_rows[None], (P, 2, SIZE)).astype(np.float32)
    )

    batch = x.shape[0]
    if batch % (N_CORES * 2 * P) != 0:
        # Shape outside the tiled layout this kernel hardcodes — fall back
        # to a host matmul (correct, just not accelerated).
        full = _compose_w1t(params)
        y_lo = x[:, :HALF].astype(np.float64) @ full
        y_hi = x[:, HALF:].astype(np.float64) @ full
        a, b = params[0][0, 0].astype(np.float64), params[0][0, 1].astype(
            np.float64
        )
        c, dd = params[0][1, 0].astype(np.float64), params[0][1, 1].astype(
            np.float64
        )
        return np.concatenate(
            [a * y_lo + b * y_hi, c * y_lo + dd * y_hi], axis=1
        ).astype(np.float32)
    rows = batch // N_CORES

    if rows not in _nc_cache:
        _nc_cache[rows] = _build_nc(rows)
    nc = _nc_cache[rows]

    in_maps = [
        {"x": x[i * rows : (i + 1) * rows], "w": w1t, "coef": coef}
        for i in range(N_CORES)
    ]
    try:
        res = run_bass_kernel_spmd(nc, in_maps, core_ids=list(range(N_CORES)))
    except Exception:
        # Transient axon/PJRT INTERNAL errors have been observed on the
        # first attempt in a fresh process; one retry clears them.
        res = run_bass_kernel_spmd(nc, in_maps, core_ids=list(range(N_CORES)))
    last_exec_time_ns = res.exec_time_ns
    last_mean_exec_time_ns = res.mean_exec_time_ns

    return np.concatenate([r["o"] for r in res.results], axis=0)


# revision 15
# speedup vs baseline: 1.2339x; 1.0926x over previous
"""Trainium2 Bass kernel for nn_Block2x2DiagProduct (butterfly product).

Strategy:
  Stages 1..9 of the butterfly (all with block size <= 512) compose into
  blockdiag(R, R) with a single dense 512x512 matrix R shared by both
  halves (parameters are shared across blocks within each factor). The
  final stage (block size 1024) is a columnwise 2x2 butterfly:

      out[:, k]     = A[k]*y[:, k] + B[k]*y[:, 512+k]
      out[:, 512+k] = C[k]*y[:, k] + D[k]*y[:, 512+k]

  where y = x @ blockdiag(R^T, R^T). So the device kernel is two K=512
  float32r matmuls per row tile (PE) plus a columnwise combine. This
  halves the PE matmul work vs composing one dense 1024x1024 matrix.

  R is composed on the host in float64. Sharding: pure data parallel —
  batch dim of x split across 8 cores; R^T (1 MiB) and the stage-0
  coefficients are replicated.

  Per-core pipeline, per 128-row tile of x:
    - HWDGE DMA in (f32 bits typed as float32r; PE rounds internally).
    - PE transposes the 8 [128,128] feature chunks 4-up into [128,512]
      PSUM tiles. Data and identity are float32r: f32r transposes run
      1.5 cyc/row vs 2.0 for plain f32 (walrus rejects mixing 32-bit
      f32r data with a 16-bit identity, so bf16's 1.0 is out of reach).
    - Scalar stages the transposes to SBUF; 8 accumulating f32r matmuls
      produce y_lo, y_hi in PSUM.
    - Stage-0 butterfly: Vector does 4 muls [128,512] against the
      (A|C) and (B|D) coefficient rows; GpSimd does ONE [128,1024] add
      (o = t02 + t13), then a per-tile 512 KiB HWDGE store.
  A burst of dummy PE transposes at t=0 starts the Tensor-engine
  p-state ramp early (cold PE runs matmuls at ~half clock for the
  first ~20 us otherwise), and the first two x loads are split into
  quarter/half transfers so real compute starts ~2 us sooner.
"""

import os
import sys

for _p in ("/opt/trn_rl_repo", "/root/.axon_site/_ro/trn_rl_repo"):
    if os.path.isdir(_p) and _p not in sys.path:
        sys.path.insert(0, _p)

import numpy as np

import concourse.bacc as bacc
import concourse.bass as bass
import concourse.mybir as mybir
from concourse.bass_utils import run_bass_kernel_spmd
from concourse.masks import make_identity
from concourse.tile import TileContext

SIZE = 1024
HALF = SIZE // 2
M = 10  # number of butterfly factors
N_CORES = 8
P = 128
KC = HALF // P  # 4 contraction chunks per half
N_WARM = 12  # dummy PE transposes to start the p-state ramp at t=0

# Results of the last device run (for the test harness).
last_exec_time_ns = None
last_mean_exec_time_ns = None

_nc_cache = {}


def _compose_w1t(params):
    """Compose butterfly stages 1..9 into W1t (512x512, f64) such that
    y_half = x_half @ W1t for each 512 half. Both halves share W1t because
    each factor's parameters are shared across its blocks."""
    w = np.eye(HALF, dtype=np.float64)
    for i in reversed(range(1, M)):
        s = SIZE >> i
        y = w.reshape(HALF, HALF // s, 2, s // 2)
        w = np.einsum(
            "ijk,bnjk->bnik", params[i].astype(np.float64), y
        ).reshape(HALF, HALF)
    return w


def _build_nc(rows):
    f32 = mybir.dt.float32
    f32r = mybir.dt.float32r
    nb = rows // P

    # Bacc (not raw Bass): its finalize() pipeline splits multi-sem waits
    # into EventSemaphore instructions (HW allows 1 sync-wait per inst).
    nc = bacc.Bacc(None, target_bir_lowering=False)
    x_d = nc.dram_tensor("x", [rows, SIZE], f32r, kind="ExternalInput")
    w_d = nc.dram_tensor("w", [HALF, HALF], f32, kind="ExternalInput")
    coef_d = nc.dram_tensor("coef", [P, 2, SIZE], f32, kind="ExternalInput")
    o_d = nc.dram_tensor("o", [rows, SIZE], f32, kind="ExternalOutput")

    with TileContext(nc) as tc:
        with (
            tc.tile_pool(name="const", bufs=1) as const_pool,
            tc.tile_pool(name="xin", bufs=8) as xpool,
            tc.tile_pool(name="xt", bufs=4) as xtpool,
            tc.tile_pool(name="tmul", bufs=6) as tpool,
            tc.tile_pool(name="osb", bufs=6) as opool,
            tc.tile_pool(name="tpsum", bufs=4, space="PSUM") as tpsum,
            tc.tile_pool(name="mpsum", bufs=4, space="PSUM") as mpsum,
        ):
            # GpSimd memset/affine_select reject f32r tiles, so build the
            # identity in f32 and cast (bit-copy) to the f32r operand the
            # transposes use. The warmup source reuses the f32r identity.
            ident_f32 = const_pool.tile([P, P], f32)
            make_identity(nc, ident_f32[:])
            ident = const_pool.tile([P, P], f32r)
            nc.vector.tensor_copy(out=ident[:], in_=ident_f32[:])
            warm_src = ident
            # PE p-state warmup: back-to-back dummy transposes from t~0.5us
            # keep the Tensor engine continuously busy so it reaches full
            # clock before the first real tile arrives (cold PE runs at
            # ~1.2 GHz, halving matmul throughput for ~20 us otherwise).
            # Also absorbs the one-sync-wait rule for the first real
            # transpose (identity is already consumed on-engine).
            pwarm = tpsum.tile([P, P], f32r, name="pwarm", tag="pst")
            for _ in range(N_WARM):
                nc.tensor.transpose(pwarm[:], warm_src[:], ident[:])

            # W1t resident in SBUF: partition p, chunk c holds W1t[c*128+p, :].
            # Per-chunk loads on the ACT HWDGE queue run in parallel with the
            # x loads on the SP queue; chunk 0's float32r cast is ready early.
            w_sb = const_pool.tile([P, KC, HALF], f32)
            w_sbr = const_pool.tile([P, KC, HALF], f32r)
            for c in range(KC):
                nc.scalar.dma_start(
                    out=w_sb[:, c, :], in_=w_d[c * P : (c + 1) * P, :]
                )
                # FP32r matmul operands must be produced rounded-to-FP32r.
                nc.vector.tensor_copy(out=w_sbr[:, c, :], in_=w_sb[:, c, :])
            # Stage-0 coefficients as (A|C) and (B|D) rows, pre-replicated
            # across partitions: o_lo|o_hi = (A|C)*y_lo_2 + (B|D)*y_hi_2.
            coef_sb = const_pool.tile([P, 2, SIZE], f32)
            nc.scalar.dma_start(out=coef_sb[:], in_=coef_d[:, :, :])

            # Store issues are deferred by two tiles: a store's semaphore
            # wait (on its GpSimd add) otherwise head-of-line blocks the
            # in-order Scalar sequencer and delays the next tile's PSUM->
            # SBUF staging copy, stalling PE. Two tiles (~5 us) is past
            # the add's completion, so the deferred wait is already
            # satisfied when the sequencer reaches it.
            pending_stores = []

            def emit_store(force=False):
                while pending_stores and (force or len(pending_stores) > 2):
                    o_prev, row0 = pending_stores.pop(0)
                    nc.scalar.dma_start(
                        out=o_d[row0 : row0 + P, :], in_=o_prev[:]
                    )

            # The per-tile compute chain is software-pipelined one tile
            # deep at EMISSION level: transposes for tile t+1 are emitted
            # before the matmuls of tile t, so the in-order PE stream
            # never sits waiting for tile t's PSUM->SBUF staging copies —
            # it transposes the next tile instead.
            def emit_compute(xts, row0):
                psos = [
                    mpsum.tile([P, HALF], f32, tag="mm_psum", name=f"pso{h}")
                    for h in range(2)
                ]
                for c in range(KC):
                    for h in range(2):
                        nc.tensor.matmul(
                            psos[h][:],
                            xts[h][:, c * P : (c + 1) * P],
                            w_sbr[:, c, :],
                            start=(c == 0),
                            stop=(c == KC - 1),
                        )
                # Peeled stage 0: Vector computes t02 = (A|C)*y_lo and
                # t13 = (B|D)*y_hi straight from PSUM (GpSimd cannot
                # read PSUM); GpSimd does one [128,1024] add into the
                # output tile, which gates the per-tile 512 KiB store.
                t02 = tpool.tile([P, SIZE], f32, tag="t02", name="t02")
                t13 = tpool.tile([P, SIZE], f32, tag="t13", name="t13")
                nc.vector.tensor_mul(
                    t02[:, :HALF], psos[0][:], coef_sb[:, 0, :HALF]
                )
                nc.vector.tensor_mul(
                    t02[:, HALF:], psos[0][:], coef_sb[:, 0, HALF:]
                )
                nc.vector.tensor_mul(
                    t13[:, :HALF], psos[1][:], coef_sb[:, 1, :HALF]
                )
                nc.vector.tensor_mul(
                    t13[:, HALF:], psos[1][:], coef_sb[:, 1, HALF:]
                )
                o_sb = opool.tile([P, SIZE], f32, tag="osb", name="osb")
                nc.gpsimd.tensor_add(o_sb[:], t02[:], t13[:])
                pending_stores.append((o_sb, row0))
                emit_store()

            pending_compute = None
            x_sb = None
            for t in range(nb):
                bp, j = divmod(t, 2)
                if j == 0:
                    # Two 128-row tiles per DMA: 1 MiB transfers are the
                    # DMA bandwidth sweet spot. The first two loads are
                    # split into quarter/half transfers so the first
                    # transposes start ~2 us earlier (shorter ramp).
                    x_sb = xpool.tile([P, 2, SIZE], f32r)
                    if bp == 0:
                        for jj in range(2):
                            for hh in range(2):
                                nc.sync.dma_start(
                                    out=x_sb[
                                        :, jj, hh * HALF : (hh + 1) * HALF
                                    ],
                                    in_=x_d[
                                        (bp * 2 + jj) * P : (bp * 2 + jj + 1)
                                        * P,
                                        hh * HALF : (hh + 1) * HALF,
                                    ],
                                )
                    elif bp == 1:
                        for jj in range(2):
                            nc.sync.dma_start(
                                out=x_sb[:, jj, :],
                                in_=x_d[
                                    (bp * 2 + jj) * P : (bp * 2 + jj + 1) * P,
                                    :,
                                ],
                            )
                    else:
                        nc.sync.dma_start(
                            out=x_sb[:],
                            in_=x_d[
                                bp * 2 * P : (bp + 1) * 2 * P, :
                            ].rearrange("(j p) f -> p j f", p=P),
                        )
                # Transpose 8 chunks of [128b, 128f] -> [128f, 128b],
                # 4 chunks per PSUM bank, one Scalar-engine copy per half.
                # The copies are emitted AFTER the previous tile's compute:
                # matmul wait thresholds coarsen to the latest Scalar op
                # emitted before them, so emitting copies(t) after mm(t-1)
                # keeps mm(t)'s effective wait on copies(t) (correct, and
                # long done) instead of copies(t+1) (a ~1 us PE stall per
                # tile observed when copies precede the compute emission).
                psts = []
                for h in range(2):
                    pst = tpsum.tile([P, HALF], f32r, tag="pst", name=f"pst{h}")
                    for c in range(KC):
                        k = KC * h + c
                        nc.tensor.transpose(
                            pst[:, c * P : (c + 1) * P],
                            x_sb[:, j, k * P : (k + 1) * P],
                            ident[:],
                        )
                    psts.append(pst)
                if pending_compute is not None:
                    emit_compute(*pending_compute)
                xts = []
                for h in range(2):
                    xt_h = xtpool.tile([P, HALF], f32r, tag="xt", name=f"xt{h}")
                    nc.scalar.copy(out=xt_h[:], in_=psts[h][:])
                    xts.append(xt_h)
                pending_compute = (xts, t * P)
            emit_compute(*pending_compute)
            emit_store(force=True)
    nc.finalize()
    return nc


def kernel(**inputs):
    global last_exec_time_ns, last_mean_exec_time_ns

    x = np.ascontiguousarray(np.asarray(inputs["x"], dtype=np.float32))
    params = [np.asarray(inputs[f"ABCD{i}"]) for i in range(M)]
    w1t = np.ascontiguousarray(_compose_w1t(params).astype(np.float32))
    abcd0 = params[0].astype(np.float32)  # (2, 2, 512)
    # coef[:, 0, :] = (A|C), coef[:, 1, :] = (B|D), replicated across
    # partitions: o = coef0 * (y_lo|y_lo) + coef1 * (y_hi|y_hi).
    coef_rows = np.stack(
        [
            np.concatenate([abcd0[0, 0], abcd0[1, 0]]),
            np.concatenate([abcd0[0, 1], abcd0[1, 1]]),
        ]
    )  # (2, 1024)
    coef = np.ascontiguousarray(
        np.broadcast_to(coef_rows[None], (P, 2, SIZE)).astype(np.float32)
    )

    batch = x.shape[0]
    if batch % (N_CORES * 2 * P) != 0:
        # Shape outside the tiled layout this kernel hardcodes — fall back
        # to a host matmul (correct, just not accelerated).
        full = _compose_w1t(params)
        y_lo = x[:, :HALF].astype(np.float64) @ full
        y_hi = x[:, HALF:].astype(np.float64) @ full
        a, b = params[0][0, 0].astype(np.float64), params[0][0, 1].astype(
            np.float64
        )
        c, dd = params[0][1, 0].astype(np.float64), params[0][1, 1].astype(
            np.float64
        )
        return np.concatenate(
            [a * y_lo + b * y_hi, c * y_lo + dd * y_hi], axis=1
        ).astype(np.float32)
    rows = batch // N_CORES

    if rows not in _nc_cache:
        _nc_cache[rows] = _build_nc(rows)
    nc = _nc_cache[rows]

    in_maps = [
        {"x": x[i * rows : (i + 1) * rows], "w": w1t, "coef": coef}
        for i in range(N_CORES)
    ]
    try:
        res = run_bass_kernel_spmd(nc, in_maps, core_ids=list(range(N_CORES)))
    except Exception:
        # Transient axon/PJRT INTERNAL errors have been observed on the
        # first attempt in a fresh process; one retry clears them.
        res = run_bass_kernel_spmd(nc, in_maps, core_ids=list(range(N_CORES)))
    last_exec_time_ns = res.exec_time_ns
    last_mean_exec_time_ns = res.mean_exec_time_ns

    return np.concatenate([r["o"] for r in res.results], axis=0)
